# revision 10
# baseline (speedup 1.0000x reference)
"""Center-loss kernel for Trainium2 (8 NeuronCores, SPMD data-parallel).

Math: with per-class sums S_c = sum_{i: l_i=c} x_i, counts N_c, and
M_c = max(N_c, 1), the reference loss

    mean((centroid[l] - x)^2)  with centroid_c = S_c / M_c

expands to

    ( sum(x^2) - sum_c ||S_c||^2 / M_c ) / (n*d)

(the N_c = 0 case contributes 0 to both forms). So one pass over the
features suffices: per-class sums + counts + global sum of squares.

Device work per core (shard of 32768 rows x 256 cols, f32):
  - DMA 4 MiB tiles [128, 32*256]  (each partition holds 256 consecutive
    rows of the shard, so every tile is 128 contiguous 32 KiB reads)
  - ACT: Square with accum_out -> per-partition running sum of x^2
  - DVE: one-hot [128, 64] per 128-sample group via is_equal(iota, label)
  - PE : psum_sums[64,256] += onehot^T @ x_group   (PSUM accumulation)
         psum_cnt [64,1]   += onehot^T @ ones
Host: reduce the tiny per-core outputs and finish the scalar in f64.
"""

import numpy as np
from contextlib import ExitStack

import concourse.bass as bass
import concourse.bacc as bacc
import concourse.mybir as mybir
import concourse.tile as tile
from concourse.bass_utils import run_bass_kernel_spmd

# Hardcoded problem shape (contract: kernel.py is self-contained).
N, D = 262144, 256
N_CLASS = 64
N_CORES = 8
NS = N // N_CORES            # 32768 rows per core
P = 128                      # SBUF partitions = contraction dim per group
GROUPS = NS // P             # 256 groups of 128 rows per core
G_PER_TILE = 32              # one DMA tile = [128, 32*256] f32 = 4 MiB
N_TILES = GROUPS // G_PER_TILE

_built = None
last_results = None          # BassKernelResults of most recent run (for test.py)


def _build(repeats=1, g_per_tile=G_PER_TILE, xbufs=3, alt_dma=False):
    n_tiles = GROUPS // g_per_tile
    nc = bacc.Bacc("TRN2", num_devices=N_CORES)
    x = nc.dram_tensor("x", [NS, D], mybir.dt.float32, kind="ExternalInput")
    lab = nc.dram_tensor("lab", [P, GROUPS], mybir.dt.float32, kind="ExternalInput")
    iota = nc.dram_tensor("iota", [P, N_CLASS], mybir.dt.float32, kind="ExternalInput")
    out_cls = nc.dram_tensor(
        "out_cls", [N_CLASS, D + 1], mybir.dt.float32, kind="ExternalOutput"
    )
    out_sq = nc.dram_tensor(
        "out_sq", [P, n_tiles], mybir.dt.float32, kind="ExternalOutput"
    )

    with ExitStack() as ctx:
        tc = ctx.enter_context(tile.TileContext(nc))
        singles = ctx.enter_context(tc.tile_pool(name="singles", bufs=1))
        xpool = ctx.enter_context(tc.tile_pool(name="xpool", bufs=xbufs))
        ohpool = ctx.enter_context(tc.tile_pool(name="ohpool", bufs=4))
        psum = ctx.enter_context(tc.tile_pool(name="psum", bufs=1, space="PSUM"))

        lab_sb = singles.tile([P, GROUPS], mybir.dt.float32)
        nc.sync.dma_start(out=lab_sb[:], in_=lab.ap())
        iota_sb = singles.tile([P, N_CLASS], mybir.dt.float32)
        nc.sync.dma_start(out=iota_sb[:], in_=iota.ap())
        ones_sb = singles.tile([P, 1], mybir.dt.float32)
        nc.vector.memset(ones_sb[:], 1.0)
        sq_acc = singles.tile([P, n_tiles], mybir.dt.float32)
        sq_scr = singles.tile([P, g_per_tile, D], mybir.dt.float32)

        ps_sums = psum.tile([N_CLASS, D], mybir.dt.float32)
        ps_cnt = psum.tile([N_CLASS, 1], mybir.dt.float32)

        # Partition p holds the shard's rows [p*256, (p+1)*256) flattened, so
        # every tile DMA is 128 contiguous 16 KiB chunks. Group gi = t*16+g is
        # sample p*256 + gi of partition p; labels arrive as the matching
        # [128, 256] = labels.reshape(128, 256) with no host transpose.
        xr = x.ap().rearrange("(p r) d -> p r d", p=P)
        for rep in range(repeats):
            for t in range(n_tiles):
                xt = xpool.tile([P, g_per_tile, D], mybir.dt.float32)
                dma_eng = nc.scalar if (alt_dma and t % 2) else nc.sync
                dma_eng.dma_start(
                    out=xt[:], in_=xr[:, t * g_per_tile : (t + 1) * g_per_tile, :]
                )
                nc.scalar.activation(
                    out=sq_scr[:],
                    in_=xt[:],
                    func=mybir.ActivationFunctionType.Square,
                    accum_out=sq_acc[:, t : t + 1],
                )
                for g in range(g_per_tile):
                    gi = t * g_per_tile + g
                    oh = ohpool.tile([P, N_CLASS], mybir.dt.float32)
                    nc.vector.tensor_scalar(
                        out=oh[:],
                        in0=iota_sb[:],
                        scalar1=lab_sb[:, gi : gi + 1],
                        scalar2=None,
                        op0=mybir.AluOpType.is_equal,
                    )
                    nc.tensor.matmul(
                        out=ps_sums[:],
                        lhsT=oh[:],
                        rhs=xt[:, g, :],
                        start=(gi == 0),
                        stop=(gi == GROUPS - 1),
                    )
                    nc.tensor.matmul(
                        out=ps_cnt[:],
                        lhsT=oh[:],
                        rhs=ones_sb[:],
                        start=(gi == 0),
                        stop=(gi == GROUPS - 1),
                    )

        out_sb = singles.tile([N_CLASS, D + 1], mybir.dt.float32)
        nc.vector.tensor_copy(out_sb[:, 0:D], ps_sums[:])
        nc.vector.tensor_copy(out_sb[:, D : D + 1], ps_cnt[:])
        nc.sync.dma_start(out=out_cls.ap(), in_=out_sb[:])
        nc.sync.dma_start(out=out_sq.ap(), in_=sq_acc[:])
    nc.compile()
    return nc


def kernel(s_feature, s_labels):
    global _built, last_results
    s_feature = np.ascontiguousarray(np.asarray(s_feature), dtype=np.float32)
    s_labels = np.asarray(s_labels)

    if _built is None:
        _built = _build()
    nc = _built

    iota_np = np.ascontiguousarray(
        np.broadcast_to(np.arange(N_CLASS, dtype=np.float32), (P, N_CLASS))
    )
    in_maps = []
    for c in range(N_CORES):
        xs = s_feature[c * NS : (c + 1) * NS]
        ls = s_labels[c * NS : (c + 1) * NS]
        lab_t = np.ascontiguousarray(np.asarray(ls).reshape(P, GROUPS).astype(np.float32))
        in_maps.append({"x": xs, "lab": lab_t, "iota": iota_np})

    try:
        last_results = run_bass_kernel_spmd(nc, in_maps, core_ids=list(range(N_CORES)))
    except ModuleNotFoundError:
        # BASS_TRACE requested but the axon NTFF hook isn't present in this
        # container; rerun with tracing hard-disabled.
        import os

        os.environ["BASS_NEVER_TRACE"] = "1"
        last_results = run_bass_kernel_spmd(nc, in_maps, core_ids=list(range(N_CORES)))

    sums = np.zeros((N_CLASS, D), dtype=np.float64)
    counts = np.zeros((N_CLASS,), dtype=np.float64)
    s2 = 0.0
    for r in last_results.results:
        oc = np.asarray(r["out_cls"], dtype=np.float64)
        sums += oc[:, :D]
        counts += oc[:, D]
        s2 += float(np.asarray(r["out_sq"], dtype=np.float64).sum())

    denom = np.maximum(counts, 1.0)
    corr = float(np.sum(np.sum(sums * sums, axis=1) / denom))
    loss = (s2 - corr) / (float(N) * float(D))
    return np.array(loss, dtype=np.float32)


# revision 22
# speedup vs baseline: 3.3000x; 3.3000x over previous
"""Center-loss kernel for Trainium2 (8 NeuronCores, SPMD data-parallel).

Math: with per-class sums S_c = sum_{i: l_i=c} x_i, counts N_c, and
M_c = max(N_c, 1), the reference loss

    mean((centroid[l] - x)^2)  with centroid_c = S_c / M_c

expands to

    ( sum(x^2) - sum_c ||S_c||^2 / M_c ) / (n*d)

(the N_c = 0 case contributes 0 to both forms). So one pass over the
features suffices: per-class sums + counts + global sum of squares.

Device work per core (shard of 32768 rows x 256 cols, f32):
  - DMA 4 MiB tiles [128, 32*256]  (each partition holds 256 consecutive
    rows of the shard, so every tile is 128 contiguous 32 KiB reads)
  - ACT: Square with accum_out -> per-partition running sum of x^2
  - DVE: one-hot [128, 64] per 128-sample group via is_equal(iota, label)
  - PE : psum_sums[64,256] += onehot^T @ x_group   (PSUM accumulation)
         psum_cnt [64,1]   += onehot^T @ ones
Host: reduce the tiny per-core outputs and finish the scalar in f64.
"""

import numpy as np
from contextlib import ExitStack

import concourse.bass as bass
import concourse.bacc as bacc
import concourse.mybir as mybir
import concourse.tile as tile
from concourse.bass_utils import run_bass_kernel_spmd

# Hardcoded problem shape (contract: kernel.py is self-contained).
N, D = 262144, 256
N_CLASS = 64
N_CORES = 8
NS = N // N_CORES            # 32768 rows per core
P = 128                      # SBUF partitions = contraction dim per group
GROUPS = NS // P             # 256 groups of 128 rows per core
G_PER_TILE = 32              # one DMA tile = [128, 32*256] f32 = 4 MiB
N_TILES = GROUPS // G_PER_TILE

_built = None
last_results = None          # BassKernelResults of most recent run (for test.py)


def _tile_schedule(g_per_tile):
    """Group counts per DMA tile: full-size tiles, then a tapered tail so the
    last tile's compute (which can't overlap any DMA) is short."""
    sched = []
    left = GROUPS
    while left > g_per_tile:
        sched.append(g_per_tile)
        left -= g_per_tile
    # taper the final full tile: 16+8+4+4 for g_per_tile=32
    while left > 4:
        half = max(4, left // 2)
        sched.append(half)
        left -= half
    while left > 0:
        sched.append(min(4, left))
        left -= min(4, left)
    return sched


def _build(repeats=1, g_per_tile=G_PER_TILE, xbufs=4, alt_dma=False, taper=True,
           dma_only=False):
    sched = _tile_schedule(g_per_tile) if taper else [g_per_tile] * (GROUPS // g_per_tile)
    n_tiles = len(sched)
    nc = bacc.Bacc("TRN2", num_devices=N_CORES)
    # x is declared float32r end-to-end: the PE's single-pass fp32 matmul
    # path (4x faster than fp32 for N>=256) requires operands produced as
    # f32r. numpy still supplies plain f32 bits (dt.np(float32r) == float32).
    x = nc.dram_tensor("x", [NS, D], mybir.dt.float32r, kind="ExternalInput")
    lab = nc.dram_tensor("lab", [P, GROUPS], mybir.dt.float32, kind="ExternalInput")
    iota = nc.dram_tensor("iota", [P, N_CLASS], mybir.dt.float32, kind="ExternalInput")
    out_cls = nc.dram_tensor(
        "out_cls", [N_CLASS, D + 1], mybir.dt.float32, kind="ExternalOutput"
    )
    out_sq = nc.dram_tensor(
        "out_sq", [P, n_tiles], mybir.dt.float32, kind="ExternalOutput"
    )

    with ExitStack() as ctx:
        tc = ctx.enter_context(tile.TileContext(nc))
        singles = ctx.enter_context(tc.tile_pool(name="singles", bufs=1))
        xpool = ctx.enter_context(tc.tile_pool(name="xpool", bufs=xbufs))
        ohpool = ctx.enter_context(tc.tile_pool(name="ohpool", bufs=4))
        psum = ctx.enter_context(tc.tile_pool(name="psum", bufs=1, space="PSUM"))

        # lab/iota go on the scalar-engine HWDGE ring so they don't delay the
        # feature DMAs queued on the sync ring.
        lab_sb = singles.tile([P, GROUPS], mybir.dt.float32)
        nc.scalar.dma_start(out=lab_sb[:], in_=lab.ap())
        iota_sb = singles.tile([P, N_CLASS], mybir.dt.float32)
        nc.scalar.dma_start(out=iota_sb[:], in_=iota.ap())
        # f32r matmul operands must be produced as f32r (rounded); memset
        # can't write f32r, so memset f32 then rounding-copy. N=2 because the
        # fp32r matmul path needs even src/dst free dims.
        ones_f32 = singles.tile([P, 2], mybir.dt.float32)
        nc.vector.memset(ones_f32[:], 1.0)
        ones_sb = singles.tile([P, 2], mybir.dt.float32r)
        nc.vector.tensor_copy(ones_sb[:], ones_f32[:])
        sq_acc = singles.tile([P, n_tiles], mybir.dt.float32)
        sq_scr = singles.tile([P, g_per_tile * D], mybir.dt.float32)

        ps_sums = psum.tile([N_CLASS, D], mybir.dt.float32)
        ps_cnt = psum.tile([N_CLASS, 2], mybir.dt.float32)

        # Partition p holds the shard's rows [p*256, (p+1)*256) flattened, so
        # every tile DMA is 128 contiguous 16 KiB chunks. Group gi = t*16+g is
        # sample p*256 + gi of partition p; labels arrive as the matching
        # [128, 256] = labels.reshape(128, 256) with no host transpose.
        xr = x.ap().rearrange("(p r) d -> p r d", p=P)
        for rep in range(repeats):
            g0 = 0
            for t, gcount in enumerate(sched):
                xt = xpool.tile([P, g_per_tile, D], mybir.dt.float32r, tag="xt")
                dma_eng = nc.scalar if (alt_dma and t % 2) else nc.sync
                dma_eng.dma_start(
                    out=xt[:, 0:gcount, :], in_=xr[:, g0 : g0 + gcount, :]
                )
                if dma_only:
                    # tiny consumer so Tile pool accounting sees a reader
                    nc.vector.tensor_copy(
                        sq_acc[:, t % n_tiles : t % n_tiles + 1],
                        xt[:, 0, 0:1].bitcast(mybir.dt.float32),
                    )
                    g0 += gcount
                    continue
                nc.scalar.activation(
                    out=sq_scr[:, 0 : gcount * D],
                    in_=xt[:, 0:gcount, :].rearrange("p g d -> p (g d)").bitcast(
                        mybir.dt.float32
                    ),
                    func=mybir.ActivationFunctionType.Square,
                    accum_out=sq_acc[:, t : t + 1],
                )
                for g in range(gcount):
                    gi = g0 + g
                    oh = ohpool.tile([P, N_CLASS], mybir.dt.float32r)
                    nc.vector.tensor_scalar(
                        out=oh[:],
                        in0=iota_sb[:],
                        scalar1=lab_sb[:, gi : gi + 1],
                        scalar2=None,
                        op0=mybir.AluOpType.is_equal,
                    )
                    # float32r: single-pass fp32 matmul (4x faster than fp32
                    # for N>=256). The stationary operand is an exact 0/1
                    # one-hot, so every product is 0*x or 1*x and the reduced
                    # multiplier precision only truncates inputs (negligible
                    # for this loss).
                    nc.tensor.matmul(
                        out=ps_sums[:],
                        lhsT=oh[:],
                        rhs=xt[:, g, :],
                        start=(gi == 0),
                        stop=(gi == GROUPS - 1),
                    )
                    nc.tensor.matmul(
                        out=ps_cnt[:],
                        lhsT=oh[:],
                        rhs=ones_sb[:],
                        start=(gi == 0),
                        stop=(gi == GROUPS - 1),
                    )
                g0 += gcount

        out_sb = singles.tile([N_CLASS, D + 1], mybir.dt.float32)
        nc.vector.tensor_copy(out_sb[:, 0:D], ps_sums[:])
        nc.vector.tensor_copy(out_sb[:, D : D + 1], ps_cnt[:, 0:1])
        nc.sync.dma_start(out=out_cls.ap(), in_=out_sb[:])
        nc.sync.dma_start(out=out_sq.ap(), in_=sq_acc[:])
    nc.compile()
    return nc


def kernel(s_feature, s_labels):
    global _built, last_results
    s_feature = np.ascontiguousarray(np.asarray(s_feature), dtype=np.float32)
    s_labels = np.asarray(s_labels)

    if _built is None:
        _built = _build()
    nc = _built

    iota_np = np.ascontiguousarray(
        np.broadcast_to(np.arange(N_CLASS, dtype=np.float32), (P, N_CLASS))
    )
    in_maps = []
    for c in range(N_CORES):
        xs = s_feature[c * NS : (c + 1) * NS]
        ls = s_labels[c * NS : (c + 1) * NS]
        lab_t = np.ascontiguousarray(np.asarray(ls).reshape(P, GROUPS).astype(np.float32))
        in_maps.append({"x": xs, "lab": lab_t, "iota": iota_np})

    try:
        last_results = run_bass_kernel_spmd(nc, in_maps, core_ids=list(range(N_CORES)))
    except ModuleNotFoundError:
        # BASS_TRACE requested but the axon NTFF hook isn't present in this
        # container; rerun with tracing hard-disabled.
        import os

        os.environ["BASS_NEVER_TRACE"] = "1"
        last_results = run_bass_kernel_spmd(nc, in_maps, core_ids=list(range(N_CORES)))

    sums = np.zeros((N_CLASS, D), dtype=np.float64)
    counts = np.zeros((N_CLASS,), dtype=np.float64)
    s2 = 0.0
    for r in last_results.results:
        oc = np.asarray(r["out_cls"], dtype=np.float64)
        sums += oc[:, :D]
        counts += oc[:, D]
        s2 += float(np.asarray(r["out_sq"], dtype=np.float64).sum())

    denom = np.maximum(counts, 1.0)
    corr = float(np.sum(np.sum(sums * sums, axis=1) / denom))
    loss = (s2 - corr) / (float(N) * float(D))
    return np.array(loss, dtype=np.float32)
